# revision 13
# baseline (speedup 1.0000x reference)
"""Bass/Tile Trainium2 kernel for nn_AttentionSACModel (data-parallel over 8 cores).

Math (per batch row b):
  own = obs[:7]; intr = obs[7:].reshape(256,5)
  q = own@Wq+bq; k = intr@Wk+bk                      (per head, flattened hd=15)
  energy = tanh(q+k); scores = (energy . v_att) * |temp|/sqrt(5)
  alpha = softmax_n(scores); wsum = alpha^T @ intr   (per head)
  context = wsum@Wv+bv; x = LN([own,context])*g+b
  out = lrelu(lrelu(x@W1+b1)@W2+b2)@Wf+bf ; concat log_std

Device mapping (natural layout: batch rows on SBUF partitions, 128/tile):
  - q+k+bias produced in PSUM by TensorE matmuls against static block-diagonal
    weights; intr^T tiles produced on-chip by PE transposes.  tanh reads PSUM
    directly on ScalarE.  sign(v_att) is folded into the q/k weights so the
    later |v_att| multiply is a plain elementwise product (tanh is odd).
  - scores: vt = energy * vrep (bf16, contiguous) then 4 adds over d.
  - softmax: no max-subtraction needed (|scores| <= |t|/sqrt(5)*max_h sum_d
    |v_att| which is tiny vs fp32 exp range); exp on ScalarE with accum_out
    giving the denominator for free.
  - wsum: 15x tensor_tensor_reduce (e_h * intr_i, sum over n) on VectorE.
  - LN stats via bn_stats/bn_aggr; rsqrt batched once (phase B) to avoid ACT
    table-set thrash; MLP via PE with on-chip transposes (phase C).
"""
import os
import sys
import numpy as np

sys.path.insert(0, "/opt/trn_rl_repo")

OWN = 7
ID = 5          # intr feature dim
NH = 3          # heads
HD = 5          # head dim
NI = 256        # n interactions
BATCH = 32768
H1, H2 = 256, 256
ADIM = 2
IN_DIM = OWN + NH * HD  # 22
SCALE = float(np.sqrt(np.float32(HD)))
LN_EPS = 1e-5
N_CORES = 8
TILE = 128
B_CORE = BATCH // N_CORES      # 4096
NT_FULL = B_CORE // TILE       # 32

# per-tile attention chunking
NCH = 16                       # n's per transpose/matmul chunk
CHROWS = NCH * ID              # 80 flat rows per chunk
CHCOLS = NCH * NH * HD         # 240 output cols per chunk
NQ = 4                         # quarters per tile
NPQ = NI // NQ                 # 64 n per quarter
CPQ = NPQ // NCH               # 4 chunks per quarter
QW = NPQ * NH * HD             # 960 qk cols per quarter

PARAM_NAMES = [
    "Wq", "bq", "Wk", "bk", "Wv", "bv", "v_att", "temperature",
    "ln_g", "ln_b", "W1", "b1", "W2", "b2", "Wf", "bf", "log_std",
]


def _prep_statics(p):
    """Host-side preprocessing of the tiny parameters into device layouts."""
    f32 = np.float32
    Wq = np.asarray(p["Wq"], f32)       # [3,7,5]
    bq = np.asarray(p["bq"], f32)       # [3,5]
    Wk = np.asarray(p["Wk"], f32)       # [3,5,5]
    bk = np.asarray(p["bk"], f32)
    Wv = np.asarray(p["Wv"], f32)
    bv = np.asarray(p["bv"], f32)
    v = np.asarray(p["v_att"], f32)     # [3,5]
    temp = abs(float(np.asarray(p["temperature"], f32).ravel()[0]))
    ln_g = np.asarray(p["ln_g"], f32)
    ln_b = np.asarray(p["ln_b"], f32)
    W1 = np.asarray(p["W1"], f32)       # [22,256]
    b1 = np.asarray(p["b1"], f32)
    W2 = np.asarray(p["W2"], f32)       # [256,256]
    b2 = np.asarray(p["b2"], f32)
    Wf = np.asarray(p["Wf"], f32)       # [256,2]
    bf = np.asarray(p["bf"], f32)
    log_std = np.asarray(p["log_std"], f32)  # [2]

    sgn = np.sign(v)                    # [3,5]
    va = np.abs(v) * (temp / SCALE)     # folded |v|*|t|/sqrt(5)

    # wkbd [81, 240]: rows (n',i) cols (n',h,d), sign-folded block diagonal;
    # row 80 pairs with a constant-ones row of intrT and adds (bq+bk)*sgn.
    wkbd = np.zeros((CHROWS + 1, CHCOLS), f32)
    for n_ in range(NCH):
        for i in range(ID):
            for h in range(NH):
                for d in range(HD):
                    wkbd[n_ * ID + i, n_ * NH * HD + h * HD + d] = Wk[h, i, d] * sgn[h, d]
    for n_ in range(NCH):
        for h in range(NH):
            for d in range(HD):
                wkbd[CHROWS, n_ * NH * HD + h * HD + d] = (bq[h, d] + bk[h, d]) * sgn[h, d]

    # qrhs [7, NI*15]: Wq replicated over n (sign folded)
    qrhs = np.zeros((OWN, NI * NH * HD), f32)
    for n in range(NI):
        for h in range(NH):
            for d in range(HD):
                c = n * NH * HD + h * HD + d
                qrhs[:OWN, c] = Wq[h, :, d] * sgn[h, d]

    # vrep [128, 3840] bf16, layout (d, h, n)
    vrep = np.zeros((TILE, HD * NH * NI), f32)
    for d in range(HD):
        for h in range(NH):
            vrep[:, d * NH * NI + h * NI: d * NH * NI + h * NI + NI] = va[h, d]

    # wvb [15, 15]: block-diag Wv (row h*5+i, col h*5+d); bias separate
    wvb = np.zeros((15, NH * HD), f32)
    for h in range(NH):
        for i in range(ID):
            for d in range(HD):
                wvb[h * ID + i, h * HD + d] = Wv[h, i, d]
    wvbias = bv.reshape(1, NH * HD).astype(f32)

    # w1aug [22, 256] = diag(ln_g) @ W1; bias row = ln_b@W1 + b1 separate
    w1aug = (ln_g[:, None] * W1).astype(f32)
    w1bias = (ln_b @ W1 + b1).reshape(1, H1).astype(f32)

    # w2pack [128, 512]: cols 0:256 = W2[0:128], cols 256:512 = W2[128:256]
    w2pack = np.concatenate([W2[:128], W2[128:]], axis=1).astype(f32)
    w2bias = b2.reshape(1, H2).astype(f32)
    # wfpack [128, 4]: cols 0:2 = Wf[0:128], 2:4 = Wf[128:256]
    wfpack = np.concatenate([Wf[:128], Wf[128:]], axis=1).astype(f32)
    wfbias = bf.reshape(1, ADIM).astype(f32)

    lsrep = np.broadcast_to(log_std.reshape(1, ADIM), (TILE, ADIM)).astype(f32).copy()
    ident = np.eye(TILE, dtype=f32)

    bf16 = np.dtype("bfloat16") if hasattr(np, "bfloat16") else None
    import ml_dtypes
    bf16 = ml_dtypes.bfloat16
    return {
        "wkbd": wkbd.astype(bf16),
        "qrhs": qrhs.astype(bf16),
        "vrep": vrep.astype(bf16),
        "wvb": wvb.astype(bf16),
        "wvbias": wvbias.astype(bf16),
        "w1aug": w1aug.astype(bf16),
        "w1bias": w1bias.astype(bf16),
        "w2pack": w2pack.astype(bf16),
        "w2bias": w2bias.astype(bf16),
        "wfpack": wfpack.astype(bf16),
        "wfbias": wfbias.astype(bf16),
        "lsrep": lsrep,
        "ident": ident,
    }


def _build_nc(nt):
    """Build the Bass module for one core processing nt tiles of 128 rows."""
    from contextlib import ExitStack
    import concourse.bass as bass
    import concourse.tile as tile
    from concourse import bacc, mybir

    f32 = mybir.dt.float32
    bf16 = mybir.dt.bfloat16
    AF = mybir.ActivationFunctionType
    OP = mybir.AluOpType

    nc = bacc.Bacc("TRN2", num_devices=N_CORES, debug=False)

    brows = nt * TILE
    obs_d = nc.dram_tensor("obs", [brows, 7 + NI * ID], f32, kind="ExternalInput").ap()
    wkbd_d = nc.dram_tensor("wkbd", [CHROWS + 1, CHCOLS], bf16, kind="ExternalInput").ap()
    qrhs_d = nc.dram_tensor("qrhs", [OWN, NI * NH * HD], bf16, kind="ExternalInput").ap()
    vrep_d = nc.dram_tensor("vrep", [TILE, HD * NH * NI], bf16, kind="ExternalInput").ap()
    wvb_d = nc.dram_tensor("wvb", [15, NH * HD], bf16, kind="ExternalInput").ap()
    wvbias_d = nc.dram_tensor("wvbias", [1, NH * HD], bf16, kind="ExternalInput").ap()
    w1aug_d = nc.dram_tensor("w1aug", [IN_DIM, H1], bf16, kind="ExternalInput").ap()
    w1bias_d = nc.dram_tensor("w1bias", [1, H1], bf16, kind="ExternalInput").ap()
    w2pack_d = nc.dram_tensor("w2pack", [128, 2 * H2], bf16, kind="ExternalInput").ap()
    w2bias_d = nc.dram_tensor("w2bias", [1, H2], bf16, kind="ExternalInput").ap()
    wfpack_d = nc.dram_tensor("wfpack", [128, 2 * ADIM], bf16, kind="ExternalInput").ap()
    wfbias_d = nc.dram_tensor("wfbias", [1, ADIM], bf16, kind="ExternalInput").ap()
    lsrep_d = nc.dram_tensor("lsrep", [TILE, ADIM], f32, kind="ExternalInput").ap()
    ident_d = nc.dram_tensor("ident", [TILE, TILE], f32, kind="ExternalInput").ap()
    out_d = nc.dram_tensor("out", [TILE, nt * 4], f32, kind="ExternalOutput").ap()

    with tile.TileContext(nc) as tc, ExitStack() as ctx:
        singles = ctx.enter_context(tc.tile_pool(name="singles", bufs=1))
        obs_pool = ctx.enter_context(tc.tile_pool(name="obs", bufs=2))
        tr_pool = ctx.enter_context(tc.tile_pool(name="tr", bufs=2))
        att_pool = ctx.enter_context(tc.tile_pool(name="att", bufs=2))
        small_pool = ctx.enter_context(tc.tile_pool(name="small", bufs=2))
        mlp_pool = ctx.enter_context(tc.tile_pool(name="mlp", bufs=2))
        qk_psum = ctx.enter_context(tc.tile_pool(name="qkps", bufs=2, space="PSUM"))
        tr_psum = ctx.enter_context(tc.tile_pool(name="trps", bufs=2, space="PSUM"))
        mm_psum = ctx.enter_context(tc.tile_pool(name="mmps", bufs=2, space="PSUM"))

        # ---- statics into SBUF -------------------------------------------------
        ident_s = singles.tile([TILE, TILE], f32)
        nc.sync.dma_start(ident_s[:], ident_d)
        identb = singles.tile([TILE, TILE], bf16)
        nc.gpsimd.dma_start(identb[:], ident_d)
        wkbd_s = singles.tile([CHROWS + 1, CHCOLS], bf16)
        nc.sync.dma_start(wkbd_s[:], wkbd_d)
        qrhs_s = singles.tile([OWN, NI * NH * HD], bf16)
        nc.sync.dma_start(qrhs_s[:], qrhs_d)
        vrep_s = singles.tile([TILE, HD * NH * NI], bf16)
        nc.sync.dma_start(vrep_s[:], vrep_d)
        wvb_s = singles.tile([15, NH * HD], bf16)
        nc.sync.dma_start(wvb_s[:], wvb_d)
        wvbias_s = singles.tile([1, NH * HD], bf16)
        nc.sync.dma_start(wvbias_s[:], wvbias_d)
        w1aug_s = singles.tile([IN_DIM, H1], bf16)
        nc.sync.dma_start(w1aug_s[:], w1aug_d)
        w1bias_s = singles.tile([1, H1], bf16)
        nc.sync.dma_start(w1bias_s[:], w1bias_d)
        w2pack_s = singles.tile([128, 2 * H2], bf16)
        nc.sync.dma_start(w2pack_s[:], w2pack_d)
        w2bias_s = singles.tile([1, H2], bf16)
        nc.sync.dma_start(w2bias_s[:], w2bias_d)
        wfpack_s = singles.tile([128, 2 * ADIM], bf16)
        nc.sync.dma_start(wfpack_s[:], wfpack_d)
        wfbias_s = singles.tile([1, ADIM], bf16)
        nc.sync.dma_start(wfbias_s[:], wfbias_d)
        lsrep_s = singles.tile([TILE, ADIM], f32)
        nc.sync.dma_start(lsrep_s[:], lsrep_d)
        ones_row = singles.tile([1, TILE], bf16)
        nc.vector.memset(ones_row[:], 1.0)
        eps_s = singles.tile([TILE, 1], f32)
        nc.vector.memset(eps_s[:], LN_EPS)

        # manual double-buffered intrT (rows 0..79 data, row 80 constant ones)
        intrT_bufs = []
        for bi in range(2):
            ib = singles.tile([96, 4 * 512], bf16, tag=f"intrT{bi}")
            nc.vector.memset(ib[64:96, :], 1.0)
            intrT_bufs.append(ib)

        # persistent accumulators across tiles
        x_all = singles.tile([TILE, nt * IN_DIM], f32)
        muvar = singles.tile([TILE, nt * 2], f32)
        rstd_all = singles.tile([TILE, nt], f32)
        sd_all = singles.tile([TILE, nt], f32)
        out_all = singles.tile([TILE, nt * 4], f32)

        # ======================= PHASE A: attention =======================
        for t in range(nt):
            obs_t = obs_pool.tile([TILE, 7 + NI * ID], bf16)
            nc.gpsimd.dma_start(obs_t[:], obs_d[t * TILE:(t + 1) * TILE, :])

            # ownT8 [8,128]: rows 0..6 own^T, row 7 ones
            ownT_ps = tr_psum.tile([OWN, TILE], bf16, tag="tr")
            nc.tensor.transpose(ownT_ps[:], obs_t[:, 0:OWN], identb[:])
            ownT7 = small_pool.tile([OWN, TILE], bf16, tag="ownT7")
            nc.vector.tensor_copy(ownT7[:], ownT_ps[:])

            # energy [128, (d,h,n)] bf16
            energy = att_pool.tile([TILE, HD * NH * NI], bf16, tag="energy")
            en_v = energy[:].rearrange("p (d h n) -> p n h d", d=HD, h=NH)

            intrT = intrT_bufs[t % 2]

            for qq in range(NQ):
                # --- transposes of intr chunks -> psum -> sbuf (bf16 cast)
                trp = tr_psum.tile([CHROWS, 512], bf16, tag="tr")
                for cc in range(CPQ):
                    c = qq * CPQ + cc
                    nc.tensor.transpose(
                        trp[:, cc * TILE:(cc + 1) * TILE],
                        obs_t[:, OWN + c * CHROWS: OWN + (c + 1) * CHROWS],
                        identb[:],
                    )
                if qq % 2 == 0:
                    nc.vector.tensor_copy(intrT[0:CHROWS, qq * 512:(qq + 1) * 512], trp[:])
                else:
                    nc.scalar.copy(intrT[0:CHROWS, qq * 512:(qq + 1) * 512], trp[:])

                # --- q + k matmuls into PSUM
                qk = qk_psum.tile([TILE, QW], f32, tag="qk")
                q0 = qq * QW
                nc.tensor.matmul(qk[:, 0:512], ownT7[:], qrhs_s[:, q0:q0 + 512],
                                 start=True, stop=False, skip_group_check=True)
                nc.tensor.matmul(qk[:, 512:QW], ownT7[:], qrhs_s[:, q0 + 512:q0 + QW],
                                 start=True, stop=False, skip_group_check=True)
                for cc in range(CPQ):
                    lhsT = intrT[0:CHROWS + 1,
                                 (qq * CPQ + cc) * TILE:(qq * CPQ + cc + 1) * TILE]
                    c0 = cc * CHCOLS
                    c1 = c0 + CHCOLS
                    last = cc == CPQ - 1
                    if c0 < 512 and c1 > 512:
                        nc.tensor.matmul(qk[:, c0:512], lhsT, wkbd_s[:, 0:512 - c0],
                                         start=False, stop=False, skip_group_check=True)
                        nc.tensor.matmul(qk[:, 512:c1], lhsT, wkbd_s[:, 512 - c0:CHCOLS],
                                         start=False, stop=last, skip_group_check=True)
                    else:
                        nc.tensor.matmul(qk[:, c0:c1], lhsT, wkbd_s[:, :],
                                         start=False, stop=last, skip_group_check=True)

                # --- tanh PSUM -> energy (strided (d,h,n) write)
                nc.scalar.activation(
                    en_v[:, qq * NPQ:(qq + 1) * NPQ, :, :],
                    qk[:, 0:QW],
                    AF.Tanh,
                )

            # --- scores: vt = energy * vrep ; s = sum_d vt
            vt = att_pool.tile([TILE, HD * NH * NI], bf16, tag="vt")
            nc.vector.tensor_mul(vt[:], energy[:], vrep_s[:])
            W = NH * NI  # 768
            s01 = att_pool.tile([TILE, W], bf16, tag="s01")
            nc.gpsimd.tensor_add(s01[:], vt[:, 0:W], vt[:, W:2 * W])
            s23 = att_pool.tile([TILE, W], bf16, tag="s23")
            nc.gpsimd.tensor_add(s23[:], vt[:, 2 * W:3 * W], vt[:, 3 * W:4 * W])
            s0123 = att_pool.tile([TILE, W], bf16, tag="s0123")
            nc.vector.tensor_add(s0123[:], s01[:], s23[:])
            sten = att_pool.tile([TILE, W], bf16, tag="sten")
            nc.vector.tensor_add(sten[:], s0123[:], vt[:, 4 * W:5 * W])

            # --- softmax (no max subtraction): e = exp(s), den = sum_n e
            e_t = att_pool.tile([TILE, W], f32, tag="e_t")
            den = small_pool.tile([TILE, NH], f32, tag="den")
            for h in range(NH):
                nc.scalar.activation(
                    e_t[:, h * NI:(h + 1) * NI],
                    sten[:, h * NI:(h + 1) * NI],
                    AF.Exp,
                    accum_out=den[:, h:h + 1],
                )
            rden = small_pool.tile([TILE, NH], f32, tag="rden")
            nc.vector.reciprocal(rden[:], den[:])

            # --- wsum via 15x tensor_tensor_reduce
            intr_v = obs_t[:, OWN:].rearrange("p (n i) -> p i n", i=ID)
            scratch = small_pool.tile([TILE, NI], f32, tag="scratch")
            wsumN = small_pool.tile([TILE, NH * ID], f32, tag="wsumN")
            for h in range(NH):
                for i in range(ID):
                    nc.vector.tensor_tensor_reduce(
                        out=scratch[:],
                        in0=e_t[:, h * NI:(h + 1) * NI],
                        in1=intr_v[:, i, :],
                        scale=1.0,
                        scalar=0.0,
                        op0=OP.mult,
                        op1=OP.add,
                        accum_out=wsumN[:, h * ID + i:h * ID + i + 1],
                    )
            for h in range(NH):
                nc.vector.tensor_scalar_mul(
                    out=wsumN[:, h * ID:(h + 1) * ID],
                    in0=wsumN[:, h * ID:(h + 1) * ID],
                    scalar1=rden[:, h:h + 1],
                )

            # --- context = wsumN @ WvBD + bv  (via transpose + matmul)
            wsumT_ps = tr_psum.tile([NH * ID, TILE], f32, tag="tr")
            nc.tensor.transpose(wsumT_ps[:], wsumN[:], ident_s[:])
            wsumT = small_pool.tile([NH * ID, TILE], bf16, tag="wsumT")
            nc.vector.tensor_copy(wsumT[:], wsumT_ps[:])
            ctx_ps = mm_psum.tile([TILE, NH * HD], f32, tag="mm")
            nc.tensor.matmul(ctx_ps[:], wsumT[:], wvb_s[:],
                             start=True, stop=False, skip_group_check=True)
            nc.tensor.matmul(ctx_ps[:], ones_row[:], wvbias_s[:],
                             start=False, stop=True, skip_group_check=True)

            # --- x assembly + LN stats
            x_t = x_all[:, t * IN_DIM:(t + 1) * IN_DIM]
            nc.vector.tensor_copy(x_t[:, 0:OWN], obs_t[:, 0:OWN])
            nc.vector.tensor_copy(x_t[:, OWN:IN_DIM], ctx_ps[:])
            st6 = small_pool.tile([TILE, 6], f32, tag="st6")
            nc.vector.bn_stats(st6[:], x_t[:, :])
            nc.vector.bn_aggr(muvar[:, 2 * t:2 * t + 2], st6[:])

        # ======================= PHASE B: rstd =======================
        muvar_v = muvar[:].rearrange("p (t two) -> p two t", two=2)
        nc.scalar.activation(sd_all[:], muvar_v[:, 1, :], AF.Sqrt, bias=eps_s[:])
        nc.vector.reciprocal(rstd_all[:], sd_all[:])

        # ======================= PHASE C: LN + MLP =======================
        for t in range(nt):
            xh = mlp_pool.tile([TILE, IN_DIM], f32, tag="xh")
            nc.vector.tensor_scalar(
                out=xh[:],
                in0=x_all[:, t * IN_DIM:(t + 1) * IN_DIM],
                scalar1=muvar[:, 2 * t:2 * t + 1],
                scalar2=rstd_all[:, t:t + 1],
                op0=OP.subtract,
                op1=OP.mult,
            )
            xhT_ps = tr_psum.tile([IN_DIM, TILE], f32, tag="tr")
            nc.tensor.transpose(xhT_ps[:], xh[:], ident_s[:])
            xhT = mlp_pool.tile([IN_DIM, TILE], bf16, tag="xhT")
            nc.vector.tensor_copy(xhT[:], xhT_ps[:])

            h1_ps = mm_psum.tile([TILE, H1], f32, tag="mm")
            nc.tensor.matmul(h1_ps[:], xhT[:], w1aug_s[:],
                             start=True, stop=False, skip_group_check=True)
            nc.tensor.matmul(h1_ps[:], ones_row[:], w1bias_s[:],
                             start=False, stop=True, skip_group_check=True)
            # leaky relu: 0.2*x + relu(0.8*x)
            r08 = mlp_pool.tile([TILE, H1], f32, tag="r08")
            nc.scalar.activation(r08[:], h1_ps[:], AF.Relu, scale=0.8)
            h1 = mlp_pool.tile([TILE, H1], f32, tag="h1")
            nc.vector.affine_then_add(h1[:], h1_ps[:], r08[:], scale=0.2, bias=0.0)

            h1T_ps = tr_psum.tile([TILE, H1], f32, tag="tr")
            nc.tensor.transpose(h1T_ps[:, 0:TILE], h1[:, 0:TILE], ident_s[:])
            nc.tensor.transpose(h1T_ps[:, TILE:H1], h1[:, TILE:H1], ident_s[:])
            h1T = mlp_pool.tile([TILE, H1], bf16, tag="h1T")
            nc.vector.tensor_copy(h1T[:], h1T_ps[:])

            h2_ps = mm_psum.tile([TILE, H2], f32, tag="mm")
            nc.tensor.matmul(h2_ps[:], h1T[:, 0:TILE], w2pack_s[:, 0:H2],
                             start=True, stop=False, skip_group_check=True)
            nc.tensor.matmul(h2_ps[:], h1T[:, TILE:H1], w2pack_s[:, H2:2 * H2],
                             start=False, stop=False, skip_group_check=True)
            nc.tensor.matmul(h2_ps[:], ones_row[:], w2bias_s[:],
                             start=False, stop=True, skip_group_check=True)
            r08b = mlp_pool.tile([TILE, H2], f32, tag="r08b")
            nc.scalar.activation(r08b[:], h2_ps[:], AF.Relu, scale=0.8)
            h2 = mlp_pool.tile([TILE, H2], f32, tag="h2")
            nc.vector.affine_then_add(h2[:], h2_ps[:], r08b[:], scale=0.2, bias=0.0)

            h2T_ps = tr_psum.tile([TILE, H2], f32, tag="tr")
            nc.tensor.transpose(h2T_ps[:, 0:TILE], h2[:, 0:TILE], ident_s[:])
            nc.tensor.transpose(h2T_ps[:, TILE:H2], h2[:, TILE:H2], ident_s[:])
            h2T = mlp_pool.tile([TILE, H2], bf16, tag="h2T")
            nc.vector.tensor_copy(h2T[:], h2T_ps[:])

            o_ps = mm_psum.tile([TILE, ADIM], f32, tag="mm")
            nc.tensor.matmul(o_ps[:], h2T[:, 0:TILE], wfpack_s[:, 0:ADIM],
                             start=True, stop=False, skip_group_check=True)
            nc.tensor.matmul(o_ps[:], h2T[:, TILE:H2], wfpack_s[:, ADIM:2 * ADIM],
                             start=False, stop=False, skip_group_check=True)
            nc.tensor.matmul(o_ps[:], ones_row[:], wfbias_s[:],
                             start=False, stop=True, skip_group_check=True)
            nc.vector.tensor_copy(out_all[:, 4 * t:4 * t + 2], o_ps[:])
            nc.vector.tensor_copy(out_all[:, 4 * t + 2:4 * t + 4], lsrep_s[:])

        nc.sync.dma_start(out_d, out_all[:])

    nc.compile()
    return nc


_CACHE = {}


def _get_nc(nt):
    if nt not in _CACHE:
        _CACHE[nt] = _build_nc(nt)
    return _CACHE[nt]


def run_cores(obs, statics, nt=NT_FULL, n_cores=N_CORES):
    """Run the bass kernel on n_cores cores; obs [n_cores*nt*128, 1287] fp32."""
    from concourse import bass_utils
    nc = _get_nc(nt)
    rows = nt * TILE
    in_maps = []
    for c in range(n_cores):
        m = {"obs": np.ascontiguousarray(obs[c * rows:(c + 1) * rows])}
        m.update(statics)
        in_maps.append(m)
    res = bass_utils.run_bass_kernel_spmd(nc, in_maps, core_ids=list(range(n_cores)))
    outs = []
    for c in range(n_cores):
        o = res.results[c]["out"]                      # [128, nt*4]
        outs.append(o.reshape(TILE, nt, 4).transpose(1, 0, 2).reshape(rows, 4))
    return np.concatenate(outs, axis=0)


def kernel(**inputs) -> np.ndarray:
    obs = np.ascontiguousarray(np.asarray(inputs["obs"], np.float32))
    statics = _prep_statics(inputs)
    return run_cores(obs, statics, NT_FULL, N_CORES).astype(np.float32)
